# revision 17
# baseline (speedup 1.0000x reference)
"""Self-contained Trainium2 Bass kernel for nn_CausalSelfAttention_59528246722829.

Sharding: 8 cores = 2 batches x 4 head-groups (4 heads / 256 channels each).
Each core computes Q/K/V projections for its head group, causal attention
(flash-style, no max subtraction -- logits are bounded ~11.6), and a partial
output projection y_g @ Wo^T[:, cols].  The host sums the 4 partials per batch
and adds the output bias (standard tensor-parallel partial-sum unshard).

Device compute dtype: bf16 matmul operands, fp32 PSUM accumulation.
Layouts (partition x free):
  xT   [128, 8, 2048]  x^T    (e on partition)        bf16
  wT   [128, 8, 256]   Wq/Wk^T (e part, c free)       bf16
  wvP  [128, 8, 260]   Wv^T padded per-head to 65     bf16
  woT  [128, 2, 1024]  Wo^T   (c part, f free)        bf16
  QT/KT[128, 2, 2048]  Q^T/K^T (c part, t free)       bf16
  Vs   [128, 16, 4, 65] V natural (t part) + ones col bf16
  ysb  [128, 2, 2048]  y^T    (c part, t free)        bf16
Attention per (head, 512-query-chunk): S^T tiles (128 keys x 512 q) on PE,
exp on ScalarE (scale=1/8 folded in), causal mask as multiplicative bf16
constant on the 4 diagonal key-tiles, P^T @ [V|1] accumulation on PE gives
y^T plus per-query softmax denominators in row 64.
"""

import numpy as np
import ml_dtypes

BF = ml_dtypes.bfloat16
B, T, E, H, DH = 2, 2048, 1024, 16, 64
P, NE, CL, NCT = 128, 8, 256, 2
ROPE_BASE = 10000.0
N_CORES = 8
CORES = [(b, g) for b in range(B) for g in range(4)]  # (batch, head-group)


def _rope_ct():
    """C^T[p, t] = cos(theta) + sin(theta), theta = t * base^(-2*(p%32)/64).

    The reference's buggy rope (d_param = n_embd slices the heads axis)
    degenerates to an elementwise multiply of Q and K by this factor.
    """
    i = (np.arange(P) % 32).astype(np.float64)
    t = np.arange(T, dtype=np.float64)
    th = t[None, :] * (ROPE_BASE ** (-2.0 * i[:, None] / DH))
    return (np.cos(th) + np.sin(th)).astype(np.float32)


def _mask_ct():
    """mask[p, oi, q] = 1 if key (p + oi*128) <= query q else 0, per diag offset."""
    m = np.zeros((P, 4, 512), np.float32)
    p = np.arange(P)[:, None]
    q = np.arange(512)[None, :]
    for oi in range(4):
        m[:, oi, :] = (q >= p + oi * 128).astype(np.float32)
    return m


def build_nc(reps=1, phases=(0, 1, 2, 3)):
    import concourse.bass as bass
    import concourse.tile as tile
    from concourse import mybir, bacc
    from contextlib import ExitStack

    f32, bf16 = mybir.dt.float32, mybir.dt.bfloat16
    Exp = mybir.ActivationFunctionType.Exp

    # Bacc (not bare Bass): its compile() pass legalizes multi-wait
    # sync_info into EventSemaphore instructions -- walrus codegen only has
    # one inline wait slot on two-operand instructions.
    nc = bacc.Bacc("TRN2")
    # All inputs pre-transposed + pre-cast to bf16 on the host: device loads
    # are plain contiguous HWDGE DMAs (no SWDGE descriptor generation, which
    # cost ~1 ms for the f32->bf16 rearranging loads).
    xbT = nc.declare_dram_parameter("xbT", [P, NE, T], bf16, isOutput=False)
    wqT = nc.declare_dram_parameter("wqT", [P, NE, CL], bf16, isOutput=False)
    wkT = nc.declare_dram_parameter("wkT", [P, NE, CL], bf16, isOutput=False)
    wvT = nc.declare_dram_parameter("wvT", [P, NE, 4, 65], bf16, isOutput=False)
    woTp = nc.declare_dram_parameter("woTp", [P, NCT, E], bf16, isOutput=False)
    bqT = nc.declare_dram_parameter("bqT", [P, NCT], f32, isOutput=False)
    bkT = nc.declare_dram_parameter("bkT", [P, NCT], f32, isOutput=False)
    bvb = nc.declare_dram_parameter("bvb", [P, 4, 65], f32, isOutput=False)
    ropec = nc.declare_dram_parameter("ropec", [P, T], bf16, isOutput=False)
    maskc = nc.declare_dram_parameter("maskc", [P, 4, 512], bf16, isOutput=False)
    outT = nc.declare_dram_parameter("outT", [E, T], f32, isOutput=True)

    with ExitStack() as octx:
        tc = octx.enter_context(tile.TileContext(nc))
        octx.enter_context(tc.tile_pool(name="dram", bufs=1, space="DRAM"))
        for _rep in range(reps):
            _emit_body(nc, tc, tile, mybir, xbT, wqT, wkT, wvT, woTp, bqT, bkT,
                       bvb, ropec, maskc, outT, phases)
    nc.compile()
    return nc


def _emit_body(nc, tc, tile, mybir, xbT, wqT, wkT, wvT, woTp, bqT, bkT, bvb,
               ropec, maskc, outT, phases):
    from contextlib import ExitStack

    f32, bf16 = mybir.dt.float32, mybir.dt.bfloat16
    Exp = mybir.ActivationFunctionType.Exp

    with ExitStack() as ctx:
        cst = ctx.enter_context(tc.tile_pool(name="cst", bufs=1))

        # ---- Phase 0: host ships transposed layouts; SWDGE casts f32->bf16
        # straight into SBUF (no DRAM bounce, no DMA-transpose) ----
        xT = cst.tile([P, NE, T], bf16)
        wT = {}
        for nm in ("q", "k"):
            wT[nm] = cst.tile([P, NE, CL], bf16, name=f"wT{nm}", tag=f"wT{nm}")
        woT = cst.tile([P, NCT, E], bf16)
        ropeS = cst.tile([P, T], bf16)
        maskS = cst.tile([P, 4, 512], bf16)
        bqS = cst.tile([P, NCT], f32)
        bkS = cst.tile([P, NCT], f32)
        bvS = cst.tile([P, 4, 65], f32)
        onesT = cst.tile([P, DH], bf16)
        wvP = cst.tile([P, NE, 4, 65], bf16)
        QT = cst.tile([P, NCT, T], bf16)
        KT = cst.tile([P, NCT, T], bf16)
        Vs = cst.tile([P, 16, 4, 65], bf16)
        ysb = cst.tile([P, NCT, T], bf16)

        if 0 in phases:
            # Spread the big loads across several DMA queues (one per issuing
            # engine) so they run in parallel; split xT per-et so phase-1
            # matmuls can start as soon as the first chunks land.
            for et in range(NE):
                q = (nc.sync, nc.scalar)[et % 2]
                q.dma_start(xT[:, et], xbT[:, et])
            nc.sync.dma_start(wT["q"][:], wqT[:])
            nc.scalar.dma_start(wT["k"][:], wkT[:])
            # wvP ships fully formed from the host (65-wide per-head blocks,
            # col 64 = 0 so the bias add -- col-64 value 1.0 -- plants the
            # ones column of V).
            nc.sync.dma_start(wvP[:], wvT[:])
            nc.scalar.dma_start(woT[:], woTp[:])
            nc.sync.dma_start(ropeS[:], ropec[:])
            nc.scalar.dma_start(maskS[:], maskc[:])
            nc.sync.dma_start(bqS[:], bqT[:])
            nc.sync.dma_start(bkS[:], bkT[:])
            nc.sync.dma_start(bvS[:], bvb[:])
            nc.vector.memset(onesT[:], 1.0)

        # ---- Phase 1: Q^T, K^T (c part, t free) and V (t part, c free) ----
        if 1 in phases:
            _emit_phase1(nc, tc, mybir, cst, xT, wT, wvP, QT, KT, Vs, bqS, bkS,
                         bvS, ropeS)
        if 2 in phases:
            _emit_phase2(nc, tc, mybir, QT, KT, Vs, ysb, maskS, onesT)
        if 3 in phases:
            _emit_phase3(nc, tc, mybir, woT, ysb, outT)


def _emit_phase1(nc, tc, mybir, cst, xT, wT, wvP, QT, KT, Vs, bqS, bkS, bvS,
                 ropeS):
    f32, bf16 = mybir.dt.float32, mybir.dt.bfloat16
    if True:
        with (
            tc.tile_pool(name="pp", bufs=4, space="PSUM") as pp,
            tc.tile_pool(name="stg1", bufs=4) as stg1,
        ):
            from concourse import mybir as _mb

            # (POOL can't read PSUM, so both epilogues live on DVE.)
            for nm, dst, bS, eng in (
                ("q", QT, bqS, nc.vector),
                ("k", KT, bkS, nc.vector),
            ):
                for ct in range(NCT):
                    for tq in range(4):
                        ps = pp.tile([P, 512], f32, tag="ps")
                        for et in range(NE):
                            nc.tensor.matmul(
                                ps[:],
                                wT[nm][:, et, ct * P : (ct + 1) * P],
                                xT[:, et, tq * 512 : (tq + 1) * 512],
                                start=(et == 0),
                                stop=(et == NE - 1),
                            )
                        # (ps + bias) * rope fused in one op; Q on DVE,
                        # K on POOL so the two epilogues run in parallel.
                        eng.scalar_tensor_tensor(
                            out=dst[:, ct, tq * 512 : (tq + 1) * 512],
                            in0=ps[:],
                            scalar=bS[:, ct : ct + 1],
                            in1=ropeS[:, tq * 512 : (tq + 1) * 512],
                            op0=_mb.AluOpType.add,
                            op1=_mb.AluOpType.mult,
                        )
            for tt in range(16):
                ps = pp.tile([P, 260], f32, tag="ps")
                for et in range(NE):
                    nc.tensor.matmul(
                        ps[:],
                        xT[:, et, tt * P : (tt + 1) * P],
                        wvP[:, et].rearrange("p h d -> p (h d)"),
                        start=(et == 0),
                        stop=(et == NE - 1),
                    )
                nc.vector.tensor_add(
                    out=Vs[:, tt],
                    in0=ps[:].rearrange("p (h d) -> p h d", h=4),
                    in1=bvS[:],
                )

def _emit_phase2(nc, tc, mybir, QT, KT, Vs, ysb, maskS, onesT):
    f32, bf16 = mybir.dt.float32, mybir.dt.bfloat16
    Exp = mybir.ActivationFunctionType.Exp
    # ---- Phase 2: causal attention per (head, query-chunk) ----
    if True:
        with (
            tc.tile_pool(name="pss", bufs=2, space="PSUM") as pss,
            tc.tile_pool(name="psy", bufs=2, space="PSUM") as psy,
            tc.tile_pool(name="ptp", bufs=3) as ptp,
            tc.tile_pool(name="rcp", bufs=2) as rcp,
        ):
            for h in range(4):
                ct, hh = h // 2, h % 2
                pb = hh * 64
                for qc in range(4):
                    nd = 4 * qc          # non-diagonal key tiles: kt < nd
                    nkt = nd + 4
                    yps = psy.tile([65, 512], f32, tag="yps")
                    groups = [
                        list(range(g0, min(g0 + 3, nd))) for g0 in range(0, nd, 3)
                    ] + [[nd, nd + 1, nd + 2], [nd + 3]]
                    for kts in groups:
                        ng = len(kts)
                        sps = pss.tile([P, 3, 512], f32, tag="sps")
                        pt = ptp.tile([P, 3, 512], bf16)
                        if kts[0] < nd:
                            for j, kt in enumerate(kts):
                                nc.tensor.matmul(
                                    sps[:, j],
                                    KT[pb : pb + 64, ct, kt * P : (kt + 1) * P],
                                    QT[pb : pb + 64, ct, qc * 512 : (qc + 1) * 512],
                                    start=True,
                                    stop=True,
                                    skip_group_check=True,
                                )
                            nc.scalar.activation(
                                pt[:, :ng], sps[:, :ng], Exp, scale=0.125
                            )
                        else:
                            # Diagonal tiles: only columns q >= oi*128 can be
                            # unmasked -- compute S and exp on that subrange,
                            # then one POOL affine_select zeroes the causal
                            # staircase AND the untouched prefix (replaces the
                            # DVE mask multiplies entirely).
                            for j, kt in enumerate(kts):
                                s = (kt - nd) * P
                                nc.tensor.matmul(
                                    sps[:, j, s:],
                                    KT[pb : pb + 64, ct, kt * P : (kt + 1) * P],
                                    QT[pb : pb + 64, ct, qc * 512 + s : (qc + 1) * 512],
                                    start=True,
                                    stop=True,
                                    skip_group_check=True,
                                )
                                nc.scalar.activation(
                                    pt[:, j, s:], sps[:, j, s:], Exp, scale=0.125
                                )
                                nc.gpsimd.affine_select(
                                    out=pt[:, j],
                                    in_=pt[:, j],
                                    pattern=[[1, 512]],
                                    compare_op=mybir.AluOpType.is_ge,
                                    fill=0.0,
                                    base=-s,
                                    channel_multiplier=-1,
                                )
                        for j, kt in enumerate(kts):
                            nc.tensor.matmul(
                                yps[:],
                                Vs[:, kt, h],
                                pt[:, j],
                                start=(kt == 0),
                                stop=(kt == nkt - 1),
                                skip_group_check=True,
                            )
                    # Evacuate PSUM base-0 -> SBUF base-0 (HW-safe), then
                    # shift rows with SBUF->SBUF copies.  The custom-DVE
                    # reciprocal only works at partition base 0 on HW, and
                    # the ones-matmul broadcast needs both operands at the
                    # same base in {0, 32, 64}.
                    stg = rcp.tile([65, 512], f32, tag="stg")
                    nc.scalar.activation(
                        stg[:], yps[:], mybir.ActivationFunctionType.Copy
                    )
                    nc.vector.tensor_copy(
                        out=ysb[pb : pb + 64, ct, qc * 512 : (qc + 1) * 512],
                        in_=stg[0:64],
                    )
                    rf = rcp.tile([1, 512], f32, tag="rf")
                    nc.vector.tensor_copy(out=rf[0:1, :], in_=stg[64:65, :])
                    rb = rcp.tile([1, 512], bf16, tag="rb")
                    rf2 = rcp.tile([1, 512], f32, tag="rf2")
                    nc.vector.reciprocal_approx_fast(out=rf2[0:1, :], in_=rf[0:1, :])
                    nc.vector.tensor_copy(out=rb[0:1, :], in_=rf2[0:1, :])
                    bps = psy.tile([64, 512], f32, tag="yps")
                    nc.tensor.matmul(
                        bps[:],
                        onesT[0:1, :],
                        rb[0:1, :],
                        start=True,
                        stop=True,
                        skip_group_check=True,
                    )
                    nc.vector.tensor_mul(
                        out=ysb[pb : pb + 64, ct, qc * 512 : (qc + 1) * 512],
                        in0=ysb[pb : pb + 64, ct, qc * 512 : (qc + 1) * 512],
                        in1=bps[:],
                    )

def _emit_phase3(nc, tc, mybir, woT, ysb, outT):
    f32 = mybir.dt.float32
    # ---- Phase 3: partial output projection out^T = Wo^T_g . y^T ----
    if True:
        with (
            tc.tile_pool(name="pso", bufs=4, space="PSUM") as pso,
            tc.tile_pool(name="ostg", bufs=4) as ostg,
        ):
            for ft in range(8):
                for tck in range(4):
                    ps = pso.tile([P, 512], f32)
                    for ct in range(NCT):
                        nc.tensor.matmul(
                            ps[:],
                            woT[:, ct, ft * P : (ft + 1) * P],
                            ysb[:, ct, tck * 512 : (tck + 1) * 512],
                            start=(ct == 0),
                            stop=(ct == NCT - 1),
                        )
                    ob = ostg.tile([P, 512], f32)
                    # ACT evacuates PSUM (it sits closer to PSUM; DVE is busy
                    # with the attention epilogues).
                    nc.scalar.activation(
                        ob[:], ps[:], mybir.ActivationFunctionType.Copy
                    )
                    nc.sync.dma_start(
                        outT[ft * P : (ft + 1) * P, tck * 512 : (tck + 1) * 512], ob[:]
                    )


def _pet(a, inner):
    """[E_like, F] -> [P, E_like/P, F] with partition-major layout, bf16."""
    e, f = a.shape
    return np.ascontiguousarray(
        a.reshape(e // P, P, f).transpose(1, 0, 2), dtype=BF
    ).reshape(P, e // P, *inner)


def make_in_maps(x, Wq, bq, Wk, bk, Wv, bv, Wo, bo):
    ropec = _rope_ct().astype(BF)
    maskc = _mask_ct().astype(BF)
    in_maps = []
    for b, g in CORES:
        cs = g * CL
        bvb = np.empty((P, 4, 65), np.float32)
        bvb[:, :, 0:64] = bv[cs : cs + CL].reshape(4, 64)[None]
        bvb[:, :, 64] = 1.0
        # Wv^T padded per-head to 65 columns (col 64 = 0; V's ones column
        # comes from the bias add).
        wvTh = Wv[cs : cs + CL].T.reshape(E, 4, DH)  # [e, h, d]
        wvP = np.zeros((E, 4, 65), np.float32)
        wvP[:, :, :DH] = wvTh
        in_maps.append(
            {
                "xbT": _pet(x[b].T, (T,)),
                "wqT": _pet(Wq[cs : cs + CL].T, (CL,)),
                "wkT": _pet(Wk[cs : cs + CL].T, (CL,)),
                "wvT": _pet(wvP.reshape(E, 4 * 65), (4, 65)),
                "woTp": _pet(Wo[:, cs : cs + CL].T, (E,)),
                "bqT": np.ascontiguousarray(
                    bq[cs : cs + CL].reshape(NCT, P).T, dtype=np.float32
                ),
                "bkT": np.ascontiguousarray(
                    bk[cs : cs + CL].reshape(NCT, P).T, dtype=np.float32
                ),
                "bvb": bvb,
                "ropec": ropec,
                "maskc": maskc,
            }
        )
    return in_maps


def assemble_output(results, bo):
    out = np.zeros((B, T, E), np.float32)
    for c, (b, g) in enumerate(CORES):
        out[b] += np.asarray(results[c]["outT"], dtype=np.float32).T
    out += np.asarray(bo, dtype=np.float32)[None, None, :]
    return out


def kernel(x, Wq, bq, Wk, bk, Wv, bv, Wo, bo, _trace=False, _trace_kwargs=None):
    from concourse.bass_utils import run_bass_kernel_spmd

    nc = build_nc()
    in_maps = make_in_maps(x, Wq, bq, Wk, bk, Wv, bv, Wo, bo)
    res = run_bass_kernel_spmd(
        nc, in_maps, list(range(N_CORES)), trace=_trace, **(_trace_kwargs or {})
    )
    out = assemble_output(res.results, bo)
    if _trace:
        return out, res
    return out



# revision 19
# speedup vs baseline: 1.1735x; 1.1735x over previous
"""Self-contained Trainium2 Bass kernel for nn_CausalSelfAttention_59528246722829.

Sharding: 8 cores = 2 batches x 4 head-groups (4 heads / 256 channels each).
Each core computes Q/K/V projections for its head group, causal attention
(flash-style, no max subtraction -- logits are bounded ~11.6), and a partial
output projection y_g @ Wo^T[:, cols].  The host sums the 4 partials per batch
and adds the output bias (standard tensor-parallel partial-sum unshard).

Device compute dtype: bf16 matmul operands, fp32 PSUM accumulation.
Layouts (partition x free):
  xT   [128, 8, 2048]  x^T    (e on partition)        bf16
  wT   [128, 8, 256]   Wq/Wk^T (e part, c free)       bf16
  wvP  [128, 8, 260]   Wv^T padded per-head to 65     bf16
  woT  [128, 2, 1024]  Wo^T   (c part, f free)        bf16
  QT/KT[128, 2, 2048]  Q^T/K^T (c part, t free)       bf16
  Vs   [128, 16, 4, 65] V natural (t part) + ones col bf16
  ysb  [128, 2, 2048]  y^T    (c part, t free)        bf16
Attention per (head, 512-query-chunk): S^T tiles (128 keys x 512 q) on PE,
exp on ScalarE (scale=1/8 folded in), causal mask as multiplicative bf16
constant on the 4 diagonal key-tiles, P^T @ [V|1] accumulation on PE gives
y^T plus per-query softmax denominators in row 64.
"""

import numpy as np
import ml_dtypes

BF = ml_dtypes.bfloat16
B, T, E, H, DH = 2, 2048, 1024, 16, 64
P, NE, CL, NCT = 128, 8, 256, 2
ROPE_BASE = 10000.0
N_CORES = 8
CORES = [(b, g) for b in range(B) for g in range(4)]  # (batch, head-group)


def _rope_ct():
    """C^T[p, t] = cos(theta) + sin(theta), theta = t * base^(-2*(p%32)/64).

    The reference's buggy rope (d_param = n_embd slices the heads axis)
    degenerates to an elementwise multiply of Q and K by this factor.
    """
    i = (np.arange(P) % 32).astype(np.float64)
    t = np.arange(T, dtype=np.float64)
    th = t[None, :] * (ROPE_BASE ** (-2.0 * i[:, None] / DH))
    return (np.cos(th) + np.sin(th)).astype(np.float32)


def _mask_ct():
    """mask[p, oi, q] = 1 if key (p + oi*128) <= query q else 0, per diag offset."""
    m = np.zeros((P, 4, 512), np.float32)
    p = np.arange(P)[:, None]
    q = np.arange(512)[None, :]
    for oi in range(4):
        m[:, oi, :] = (q >= p + oi * 128).astype(np.float32)
    return m


def build_nc(reps=1, phases=(0, 1, 2, 3)):
    import concourse.bass as bass
    import concourse.tile as tile
    from concourse import mybir, bacc
    from contextlib import ExitStack

    f32, bf16 = mybir.dt.float32, mybir.dt.bfloat16
    Exp = mybir.ActivationFunctionType.Exp

    # Bacc (not bare Bass): its compile() pass legalizes multi-wait
    # sync_info into EventSemaphore instructions -- walrus codegen only has
    # one inline wait slot on two-operand instructions.
    nc = bacc.Bacc("TRN2")
    # All inputs pre-transposed + pre-cast to bf16 on the host: device loads
    # are plain contiguous HWDGE DMAs (no SWDGE descriptor generation, which
    # cost ~1 ms for the f32->bf16 rearranging loads).
    xbT = nc.declare_dram_parameter("xbT", [P, NE, T], bf16, isOutput=False)
    wqT = nc.declare_dram_parameter("wqT", [P, NE, CL], bf16, isOutput=False)
    wkT = nc.declare_dram_parameter("wkT", [P, NE, CL], bf16, isOutput=False)
    wvT = nc.declare_dram_parameter("wvT", [P, NE, 4, 65], bf16, isOutput=False)
    woTp = nc.declare_dram_parameter("woTp", [P, NCT, E], bf16, isOutput=False)
    bqT = nc.declare_dram_parameter("bqT", [P, NCT], f32, isOutput=False)
    bkT = nc.declare_dram_parameter("bkT", [P, NCT], f32, isOutput=False)
    bvb = nc.declare_dram_parameter("bvb", [P, 4, 65], f32, isOutput=False)
    ropec = nc.declare_dram_parameter("ropec", [P, T], bf16, isOutput=False)
    outT = nc.declare_dram_parameter("outT", [E, T], bf16, isOutput=True)

    with ExitStack() as octx:
        tc = octx.enter_context(tile.TileContext(nc))
        octx.enter_context(tc.tile_pool(name="dram", bufs=1, space="DRAM"))
        for _rep in range(reps):
            _emit_body(nc, tc, tile, mybir, xbT, wqT, wkT, wvT, woTp, bqT, bkT,
                       bvb, ropec, outT, phases)
    nc.compile()
    return nc


def _emit_body(nc, tc, tile, mybir, xbT, wqT, wkT, wvT, woTp, bqT, bkT, bvb,
               ropec, outT, phases):
    from contextlib import ExitStack

    f32, bf16 = mybir.dt.float32, mybir.dt.bfloat16
    Exp = mybir.ActivationFunctionType.Exp

    with ExitStack() as ctx:
        cst = ctx.enter_context(tc.tile_pool(name="cst", bufs=1))

        # ---- Phase 0: host ships transposed layouts; SWDGE casts f32->bf16
        # straight into SBUF (no DRAM bounce, no DMA-transpose) ----
        xT = cst.tile([P, NE, T], bf16)
        wT = {}
        for nm in ("q", "k"):
            wT[nm] = cst.tile([P, NE, CL], bf16, name=f"wT{nm}", tag=f"wT{nm}")
        woT = cst.tile([P, NCT, E], bf16)
        ropeS = cst.tile([P, T], bf16)
        bqS = cst.tile([P, NCT], f32)
        bkS = cst.tile([P, NCT], f32)
        bvS = cst.tile([P, 4, 65], f32)
        onesT = cst.tile([P, DH], bf16)
        wvP = cst.tile([P, NE, 4, 65], bf16)
        QT = cst.tile([P, NCT, T], bf16)
        KT = cst.tile([P, NCT, T], bf16)
        Vs = cst.tile([P, 16, 4, 65], bf16)
        ysb = cst.tile([P, NCT, T], bf16)

        if 0 in phases:
            # Spread the big loads across several DMA queues (one per issuing
            # engine) so they run in parallel; split xT per-et so phase-1
            # matmuls can start as soon as the first chunks land.
            for et in range(NE):
                q = (nc.sync, nc.scalar)[et % 2]
                q.dma_start(xT[:, et], xbT[:, et])
            nc.sync.dma_start(wT["q"][:], wqT[:])
            nc.scalar.dma_start(wT["k"][:], wkT[:])
            # wvP ships fully formed from the host (65-wide per-head blocks,
            # col 64 = 0 so the bias add -- col-64 value 1.0 -- plants the
            # ones column of V).
            nc.sync.dma_start(wvP[:], wvT[:])
            nc.scalar.dma_start(woT[:], woTp[:])
            nc.sync.dma_start(ropeS[:], ropec[:])
            nc.sync.dma_start(bqS[:], bqT[:])
            nc.sync.dma_start(bkS[:], bkT[:])
            nc.sync.dma_start(bvS[:], bvb[:])
            nc.vector.memset(onesT[:], 1.0)

        # ---- Phase 1: Q^T, K^T (c part, t free) and V (t part, c free) ----
        if 1 in phases:
            _emit_phase1(nc, tc, mybir, cst, xT, wT, wvP, QT, KT, Vs, bqS, bkS,
                         bvS, ropeS)
        if 2 in phases:
            _emit_phase2(nc, tc, mybir, QT, KT, Vs, ysb, onesT)
        if 3 in phases:
            _emit_phase3(nc, tc, mybir, woT, ysb, outT)


def _emit_phase1(nc, tc, mybir, cst, xT, wT, wvP, QT, KT, Vs, bqS, bkS, bvS,
                 ropeS):
    f32, bf16 = mybir.dt.float32, mybir.dt.bfloat16
    if True:
        with (
            tc.tile_pool(name="pp", bufs=4, space="PSUM") as pp,
            tc.tile_pool(name="stg1", bufs=4) as stg1,
        ):
            from concourse import mybir as _mb

            # (POOL can't read PSUM, so both epilogues live on DVE.)
            for nm, dst, bS, eng in (
                ("q", QT, bqS, nc.vector),
                ("k", KT, bkS, nc.vector),
            ):
                for ct in range(NCT):
                    for tq in range(4):
                        ps = pp.tile([P, 512], f32, tag="ps")
                        for et in range(NE):
                            nc.tensor.matmul(
                                ps[:],
                                wT[nm][:, et, ct * P : (ct + 1) * P],
                                xT[:, et, tq * 512 : (tq + 1) * 512],
                                start=(et == 0),
                                stop=(et == NE - 1),
                            )
                        # (ps + bias) * rope fused in one op; Q on DVE,
                        # K on POOL so the two epilogues run in parallel.
                        eng.scalar_tensor_tensor(
                            out=dst[:, ct, tq * 512 : (tq + 1) * 512],
                            in0=ps[:],
                            scalar=bS[:, ct : ct + 1],
                            in1=ropeS[:, tq * 512 : (tq + 1) * 512],
                            op0=_mb.AluOpType.add,
                            op1=_mb.AluOpType.mult,
                        )
            for tt in range(16):
                ps = pp.tile([P, 260], f32, tag="ps")
                for et in range(NE):
                    nc.tensor.matmul(
                        ps[:],
                        xT[:, et, tt * P : (tt + 1) * P],
                        wvP[:, et].rearrange("p h d -> p (h d)"),
                        start=(et == 0),
                        stop=(et == NE - 1),
                    )
                nc.vector.tensor_add(
                    out=Vs[:, tt],
                    in0=ps[:].rearrange("p (h d) -> p h d", h=4),
                    in1=bvS[:],
                )

def _emit_phase2(nc, tc, mybir, QT, KT, Vs, ysb, onesT):
    f32, bf16 = mybir.dt.float32, mybir.dt.bfloat16
    Exp = mybir.ActivationFunctionType.Exp
    # ---- Phase 2: causal attention per (head, query-chunk) ----
    if True:
        with (
            tc.tile_pool(name="pss", bufs=2, space="PSUM") as pss,
            tc.tile_pool(name="psy", bufs=2, space="PSUM") as psy,
            tc.tile_pool(name="ptp", bufs=6) as ptp,
            tc.tile_pool(name="rcp", bufs=2) as rcp,
        ):
            for h in range(4):
                ct, hh = h // 2, h % 2
                pb = hh * 64
                for qc in range(4):
                    nd = 4 * qc          # non-diagonal key tiles: kt < nd
                    nkt = nd + 4
                    yps = psy.tile([65, 512], f32, tag="yps")
                    groups = [
                        list(range(g0, min(g0 + 3, nd))) for g0 in range(0, nd, 3)
                    ] + [[nd, nd + 1, nd + 2], [nd + 3]]
                    for kts in groups:
                        ng = len(kts)
                        sps = pss.tile([P, 3, 512], f32, tag="sps")
                        pt = ptp.tile([P, 3, 512], bf16)
                        if kts[0] < nd:
                            for j, kt in enumerate(kts):
                                nc.tensor.matmul(
                                    sps[:, j],
                                    KT[pb : pb + 64, ct, kt * P : (kt + 1) * P],
                                    QT[pb : pb + 64, ct, qc * 512 : (qc + 1) * 512],
                                    start=True,
                                    stop=True,
                                    skip_group_check=True,
                                )
                            nc.scalar.activation(
                                pt[:, :ng], sps[:, :ng], Exp, scale=0.125
                            )
                        else:
                            # Diagonal tiles: only columns q >= oi*128 can be
                            # unmasked -- compute S and exp on that subrange,
                            # then one POOL affine_select zeroes the causal
                            # staircase AND the untouched prefix (replaces the
                            # DVE mask multiplies entirely).
                            for j, kt in enumerate(kts):
                                s = (kt - nd) * P
                                nc.tensor.matmul(
                                    sps[:, j, s:],
                                    KT[pb : pb + 64, ct, kt * P : (kt + 1) * P],
                                    QT[pb : pb + 64, ct, qc * 512 + s : (qc + 1) * 512],
                                    start=True,
                                    stop=True,
                                    skip_group_check=True,
                                )
                                nc.scalar.activation(
                                    pt[:, j, s:], sps[:, j, s:], Exp, scale=0.125
                                )
                                nc.gpsimd.affine_select(
                                    out=pt[:, j],
                                    in_=pt[:, j],
                                    pattern=[[1, 512]],
                                    compare_op=mybir.AluOpType.is_ge,
                                    fill=0.0,
                                    base=-s,
                                    channel_multiplier=-1,
                                )
                        for j, kt in enumerate(kts):
                            nc.tensor.matmul(
                                yps[:],
                                Vs[:, kt, h],
                                pt[:, j],
                                start=(kt == 0),
                                stop=(kt == nkt - 1),
                                skip_group_check=True,
                            )
                    # Evacuate PSUM base-0 -> SBUF base-0 (HW-safe), then
                    # shift rows with SBUF->SBUF copies.  The custom-DVE
                    # reciprocal only works at partition base 0 on HW, and
                    # the ones-matmul broadcast needs both operands at the
                    # same base in {0, 32, 64}.
                    stg = rcp.tile([65, 512], f32, tag="stg")
                    nc.scalar.activation(
                        stg[:], yps[:], mybir.ActivationFunctionType.Copy
                    )
                    nc.vector.tensor_copy(
                        out=ysb[pb : pb + 64, ct, qc * 512 : (qc + 1) * 512],
                        in_=stg[0:64],
                    )
                    rf = rcp.tile([1, 512], f32, tag="rf")
                    nc.vector.tensor_copy(out=rf[0:1, :], in_=stg[64:65, :])
                    rb = rcp.tile([1, 512], bf16, tag="rb")
                    rf2 = rcp.tile([1, 512], f32, tag="rf2")
                    nc.vector.reciprocal_approx_fast(out=rf2[0:1, :], in_=rf[0:1, :])
                    nc.vector.tensor_copy(out=rb[0:1, :], in_=rf2[0:1, :])
                    bps = psy.tile([64, 512], f32, tag="yps")
                    nc.tensor.matmul(
                        bps[:],
                        onesT[0:1, :],
                        rb[0:1, :],
                        start=True,
                        stop=True,
                        skip_group_check=True,
                    )
                    nc.vector.tensor_mul(
                        out=ysb[pb : pb + 64, ct, qc * 512 : (qc + 1) * 512],
                        in0=ysb[pb : pb + 64, ct, qc * 512 : (qc + 1) * 512],
                        in1=bps[:],
                    )

def _emit_phase3(nc, tc, mybir, woT, ysb, outT):
    f32, bf16 = mybir.dt.float32, mybir.dt.bfloat16
    # ---- Phase 3: partial output projection out^T = Wo^T_g . y^T ----
    if True:
        with (
            tc.tile_pool(name="pso", bufs=4, space="PSUM") as pso,
            tc.tile_pool(name="ostg", bufs=4) as ostg,
        ):
            for ft in range(8):
                for tck in range(4):
                    ps = pso.tile([P, 512], f32)
                    for ct in range(NCT):
                        nc.tensor.matmul(
                            ps[:],
                            woT[:, ct, ft * P : (ft + 1) * P],
                            ysb[:, ct, tck * 512 : (tck + 1) * 512],
                            start=(ct == 0),
                            stop=(ct == NCT - 1),
                        )
                    ob = ostg.tile([P, 512], bf16)
                    # ACT evacuates PSUM (it sits closer to PSUM; DVE is busy
                    # with the attention epilogues).
                    nc.scalar.activation(
                        ob[:], ps[:], mybir.ActivationFunctionType.Copy
                    )
                    nc.sync.dma_start(
                        outT[ft * P : (ft + 1) * P, tck * 512 : (tck + 1) * 512], ob[:]
                    )


def _pet(a, inner):
    """[E_like, F] -> [P, E_like/P, F] with partition-major layout, bf16."""
    e, f = a.shape
    return np.ascontiguousarray(
        a.reshape(e // P, P, f).transpose(1, 0, 2), dtype=BF
    ).reshape(P, e // P, *inner)


def make_in_maps(x, Wq, bq, Wk, bk, Wv, bv, Wo, bo):
    ropec = _rope_ct().astype(BF)
    in_maps = []
    for b, g in CORES:
        cs = g * CL
        bvb = np.empty((P, 4, 65), np.float32)
        bvb[:, :, 0:64] = bv[cs : cs + CL].reshape(4, 64)[None]
        bvb[:, :, 64] = 1.0
        # Wv^T padded per-head to 65 columns (col 64 = 0; V's ones column
        # comes from the bias add).
        wvTh = Wv[cs : cs + CL].T.reshape(E, 4, DH)  # [e, h, d]
        wvP = np.zeros((E, 4, 65), np.float32)
        wvP[:, :, :DH] = wvTh
        in_maps.append(
            {
                "xbT": _pet(x[b].T, (T,)),
                "wqT": _pet(Wq[cs : cs + CL].T, (CL,)),
                "wkT": _pet(Wk[cs : cs + CL].T, (CL,)),
                "wvT": _pet(wvP.reshape(E, 4 * 65), (4, 65)),
                "woTp": _pet(Wo[:, cs : cs + CL].T, (E,)),
                "bqT": np.ascontiguousarray(
                    bq[cs : cs + CL].reshape(NCT, P).T, dtype=np.float32
                ),
                "bkT": np.ascontiguousarray(
                    bk[cs : cs + CL].reshape(NCT, P).T, dtype=np.float32
                ),
                "bvb": bvb,
                "ropec": ropec,
            }
        )
    return in_maps


def assemble_output(results, bo):
    out = np.zeros((B, T, E), np.float32)
    for c, (b, g) in enumerate(CORES):
        out[b] += np.asarray(results[c]["outT"], dtype=np.float32).T
    out += np.asarray(bo, dtype=np.float32)[None, None, :]
    return out


def kernel(x, Wq, bq, Wk, bk, Wv, bv, Wo, bo, _trace=False, _trace_kwargs=None):
    from concourse.bass_utils import run_bass_kernel_spmd

    nc = build_nc()
    in_maps = make_in_maps(x, Wq, bq, Wk, bk, Wv, bv, Wo, bo)
    res = run_bass_kernel_spmd(
        nc, in_maps, list(range(N_CORES)), trace=_trace, **(_trace_kwargs or {})
    )
    out = assemble_output(res.results, bo)
    if _trace:
        return out, res
    return out

